# revision 21
# baseline (speedup 1.0000x reference)
"""Trainium2 Bass kernel for nn_CausalSelfAttention_59253368815644.

Sharding: 8 cores = 2 (batch) x 4 (head groups of 4 heads). Per core:
qkv projection (bf16 matmuls, FWL), rms-norm via DVE squares + PE
partition-sums + ACT ln/exp (single activation table set), head-batched
rotary in bf16 on DVE (normalize-first; KEY_OFFSET via split-destination
writes), doc-masked causal attention (one-hot augmented contraction
rows; causal via affine_select), softmax without max-subtraction, gated
value embedding, attention output gate (sigmoid via exp + fast
reciprocal), partial output projection in bf16 spread over 3 DMA
queues. Host sums 4 partials per batch element.
"""
import sys

sys.path.insert(0, "/opt/trn_rl_repo")

from contextlib import ExitStack

import ml_dtypes
import numpy as np

import concourse.bass as bass
import concourse.tile as tile
from concourse import bacc, mybir
from concourse._compat import with_exitstack
from concourse.bass_utils import run_bass_kernel_spmd

F32 = mybir.dt.float32
BF16 = mybir.dt.bfloat16
AF = mybir.ActivationFunctionType
BF = ml_dtypes.bfloat16

B, T, D, H, HD = 2, 2048, 1024, 16, 64
EPS = 1.1920929e-07
VE_GATE_SCALE = 2.0
NHEADS = 4          # heads per core
HGROUPS = 4
NCHUNK = D // 128   # 8 contraction chunks
TTILE = 512
NTT = T // TTILE
BIG = 30.0          # mask exponent after exp-scale
NDOC = 8
AUG = NDOC + 1
QR = 64 + AUG       # 73 partitions for Q^/K^

# ---------------------------------------------------------------------------
# Activation-table selection patch: the greedy chooser picks the FIRST
# act_func_set containing each function (exp_and_others for Copy/Exp/Square,
# natural_log for Ln), thrashing the single resident table ~9x per kernel
# (1.28us each on the scalar queue).  natural_log_exp_and_others genuinely
# contains all four functions, so hide them from every other set; set ids
# (insertion order) are preserved so walrus's act.json emission stays
# consistent.
from concourse import hw_specs as _hw_specs
import concourse.bacc as _bacc_mod

_ORIG_GAT = _hw_specs.get_activation_tables
_SUPERSET = "natural_log_exp_and_others"
_FOLD = {AF.Ln, AF.Exp, AF.Copy, AF.Square, AF.Identity}


def _patched_gat(arch):
    tabs = _ORIG_GAT(arch)
    if _SUPERSET not in tabs or not (_FOLD <= tabs[_SUPERSET]):
        return tabs
    return {
        name: (set(funcs) if name == _SUPERSET else set(funcs) - _FOLD)
        for name, funcs in tabs.items()
    }


_bacc_mod.get_activation_tables = _patched_gat


def build_spans(segs):
    """Fixed 512-token q-spans (fewest span epilogues; for these doc
    layouts the k-tile count and streamed column total match the greedy
    doc-aligned spans). Returns [(a, b, kts)]."""
    spans = []
    for a in range(0, T, 512):
        ks = max((s for (s, _) in segs if s <= a), default=0)
        spans.append((a, min(a + 512, T), ks))
    def docend(pos):
        for (s_, e_) in segs:
            if s_ <= pos < e_:
                return e_
        return T

    out = []
    for (a, b, ks) in spans:
        ka0 = (ks // 128) * 128
        kts = []
        ka = ka0
        while ka < b:
            kn = min(128, b - ka)
            # q-columns beyond the last key row's document are fully masked
            qhi = min(b - a, docend(ka + kn - 1) - a)
            w0 = max(0, ka - a)
            if qhi > w0:
                kts.append((ka, kn, (ka + kn) > a, qhi))
            ka += 128
        out.append((a, b, kts))
    return out


@with_exitstack
def build_kernel(ctx: ExitStack, tc: tile.TileContext, dr, spans, alpha):
    nc = tc.nc

    const = ctx.enter_context(tc.tile_pool(name="const", bufs=1))
    persist = ctx.enter_context(tc.tile_pool(name="persist", bufs=1))

    # ---- constants / inputs staged in SBUF ----
    # DMA priority: the first qk matmuls need wqk chunk c + x tile0 chunk c;
    # issue those chunk-granular on two queues so compute starts as soon as
    # chunk 0 lands.  Small constants go first on the gpsimd queue; big
    # late-use tensors (vesb, wo) are deferred.
    wqk = const.tile([128, NCHUNK, 512], BF16)
    xall = const.tile([128, NTT, NCHUNK, TTILE], BF16)
    xr = dr["xT"][:].rearrange("p (u c t) -> p u c t", c=NCHUNK, t=TTILE)
    wqkr = dr["wqk"][:].rearrange("p (c e) -> p c e", e=512)
    # DMA issue costs ~0.6-1.4us per dma_start and only 3 engines can
    # issue, so use a few medium-grained transfers: first chunks of wqk +
    # x tile0 land first on separate queues, rest follows immediately
    nc.scalar.dma_start(wqk[:, 0:2, :], wqkr[:, 0:2, :])
    nc.sync.dma_start(xall[:, 0, 0:2, :], xr[:, 0, 0:2, :])
    nc.gpsimd.dma_start(xall[:, 0, 2:5, :], xr[:, 0, 2:5, :])
    nc.scalar.dma_start(xall[:, 0, 5:8, :], xr[:, 0, 5:8, :])
    nc.sync.dma_start(wqk[:, 2:8, :], wqkr[:, 2:8, :])
    # small constants next on gpsimd queue
    onesEt = const.tile([128, NHEADS, 8], BF16)
    nc.gpsimd.dma_start(onesEt[:],
                        dr["onesEt"][:].rearrange("p (b e) -> p b e", e=8))
    e8sel = const.tile([8, NHEADS, 128], BF16)
    nc.gpsimd.dma_start(e8sel[:],
                        dr["e8sel"][:].rearrange("p (b e) -> p b e", e=128))
    e4a = const.tile([4, 128], BF16)
    nc.gpsimd.dma_start(e4a[:], dr["e4"][0, :, :])
    e4b = const.tile([4, 128], BF16)
    nc.gpsimd.dma_start(e4b[:], dr["e4"][1, :, :])
    qaug = dr["qaug"]
    kaug = dr["kaug"]
    Qh = persist.tile([QR, NHEADS, T], BF16)
    Kh = persist.tile([QR, NHEADS, T], BF16)
    nc.gpsimd.dma_start(
        Qh[64:QR, :, :],
        bass.AP(tensor=qaug.tensor, offset=qaug.offset,
                ap=[[T, AUG], [0, NHEADS], [1, T]]))
    nc.gpsimd.dma_start(
        Kh[64:QR, :, :],
        bass.AP(tensor=kaug.tensor, offset=kaug.offset,
                ap=[[T, AUG], [0, NHEADS], [1, T]]))
    wv = const.tile([128, NCHUNK, 260], BF16)
    nc.gpsimd.dma_start(wv[:],
                        dr["wv"][:].rearrange("p (c e) -> p c e", e=260))
    wga36 = const.tile([128, NCHUNK, 36], BF16)
    nc.gpsimd.dma_start(wga36[:],
                        dr["wga36"][:].rearrange("p (c e) -> p c e", e=36))
    cdup = const.tile([128, T], BF16)
    nc.gpsimd.dma_start(cdup[:], dr["cdup"][:])
    s2dup = const.tile([128, T], BF16)
    nc.gpsimd.dma_start(s2dup[:], dr["s2dup"][:])
    # x tiles 1..3: tile1+wqk done ~12.5us; tt1 needs tile1 ~25us in
    nc.scalar.dma_start(xall[:, 1, :, :], xr[:, 1, :, :])
    nc.sync.dma_start(xall[:, 2, :, :], xr[:, 2, :, :])
    vesb = const.tile([128, T // 128, 256], BF16)
    nc.gpsimd.dma_start(
        vesb[:], dr["veb"][:].rearrange("p (s e) -> p s e", e=256))
    nc.scalar.dma_start(xall[:, 3, :, :], xr[:, 3, :, :])
    epsb = const.tile([8, 1], F32)
    nc.vector.memset(epsb[:], EPS)
    wo = const.tile([128, 2, 1024], BF16)
    nc.sync.dma_start(wo[:],
                      dr["wo"][:].rearrange("p (c e) -> p c e", e=1024))

    # ---- persistent activations ----
    Vh = persist.tile([128, T // 128, NHEADS, 65], BF16)
    nc.vector.memset(Vh[:, :, :, 64:65], 1.0)
    agrow1 = persist.tile([NHEADS, T], BF16)  # 1 + exp(-attn_gate_logit)

    # PE warm-up: the HAM clock gate keeps the PE at 1.2 GHz until it has
    # been busy ~3.4us; the input DMA ramp (~10us) would otherwise leave
    # the first real matmul burst cold.  Burn scratch matmuls (never read)
    # so the array is at 2.4 GHz when tile 0 lands.
    junk = persist.tile([128, 512], BF16)
    nc.vector.memset(junk[:], 0.0)
    with tc.tile_pool(name="warm", bufs=1, space="PSUM") as wps_pool:
        wps = wps_pool.tile([128, 512], F32)
        for i in range(28):
            nc.tensor.matmul(wps[:], junk[:, 0:128], junk[:],
                             start=True, stop=True)

    # =========== Phase 1 ===========
    # last 512-token tile split in two so its DVE rotary tail is half as
    # long (the tail has no tensor work left to overlap with)
    TILES = [(0, 0, 512), (1, 0, 512), (2, 0, 512), (3, 0, 256), (3, 256, 256)]
    with tc.tile_pool(name="p1qk", bufs=2, space="PSUM") as qkps_pool, \
         tc.tile_pool(name="p1ss", bufs=1, space="PSUM") as ssps_pool, \
         tc.tile_pool(name="p1v", bufs=2, space="PSUM") as vps_pool, \
         tc.tile_pool(name="p1rb", bufs=1, space="PSUM") as rbps_pool, \
         tc.tile_pool(name="p1sb", bufs=2) as sb_pool, \
         tc.tile_pool(name="p1n", bufs=2) as n1_pool:
        for (tt, off, tlen) in TILES:
            t0 = tt * TTILE + off

            # qkv projection, 2 heads per psum tile (2 banks x 2 bufs)
            qkb = sb_pool.tile([128, NHEADS, TTILE], BF16, tag="qkb")
            for g in range(2):
                qk2 = qkps_pool.tile([128, 2, TTILE], F32, tag="qk")
                for c in range(NCHUNK):
                    for j in range(2):
                        blk = 2 * g + j
                        nc.tensor.matmul(
                            qk2[:, j, 0:tlen],
                            wqk[:, c, blk * 128:(blk + 1) * 128],
                            xall[:, tt, c, off:off + tlen],
                            start=(c == 0), stop=(c == NCHUNK - 1))
                nc.scalar.activation(out=qkb[:, 2 * g:2 * g + 2, 0:tlen],
                                     in_=qk2[:, :, 0:tlen], func=AF.Copy,
                                     scale=1.0)
            sq = sb_pool.tile([128, NHEADS, TTILE], BF16, tag="sq")
            nc.scalar.activation(out=sq[:, :, 0:tlen], in_=qkb[:, :, 0:tlen],
                                 func=AF.Square, scale=1.0)

            # rotary halves (rstd applied after; its chain overlaps these)
            cb = cdup[:, t0:t0 + tlen].unsqueeze(1).broadcast_to(
                (128, NHEADS, tlen))
            sbr = s2dup[:, t0:t0 + tlen].unsqueeze(1).broadcast_to(
                (128, NHEADS, tlen))
            A = sb_pool.tile([128, NHEADS, TTILE], BF16, tag="A")
            nc.vector.tensor_mul(A[:, :, 0:tlen], qkb[:, :, 0:tlen], cb)
            Bt = sb_pool.tile([128, NHEADS, TTILE], BF16, tag="B")
            nc.vector.tensor_mul(Bt[:, :, 0:tlen], qkb[:, :, 0:tlen], sbr)
            Bs = sb_pool.tile([128, NHEADS, TTILE], BF16, tag="Bs")
            nc.vector.stream_shuffle(
                Bs[:].rearrange("p h t -> p (h t)").bitcast(mybir.dt.int32),
                Bt[:].rearrange("p h t -> p (h t)").bitcast(mybir.dt.int32),
                mask=[g ^ 16 for g in range(32)])
            rotr = sb_pool.tile([128, NHEADS, TTILE], BF16, tag="rotr")
            nc.vector.tensor_add(rotr[:, :, 0:tlen], A[:, :, 0:tlen],
                                 Bs[:, :, 0:tlen])

            # attn-gate logits (rows 32:36); per-head sum-squares rows 0:8
            ss8z = ssps_pool.tile([36, TTILE], F32, tag="ss")
            for c in range(NCHUNK):
                nc.tensor.matmul(ss8z[:, 0:tlen], wga36[:, c, :],
                                 xall[:, tt, c, off:off + tlen],
                                 start=(c == 0), stop=False)

            # value projection + gated ve (tensor-queue filler while DVE
            # computes sq/rotary and the rstd chain completes)
            for sub in range(tlen // 128):
                stg = (t0 + sub * 128) // 128
                vps = vps_pool.tile([128, 260], F32, tag="v")
                for c in range(NCHUNK):
                    nc.tensor.matmul(
                        vps[:],
                        xall[:, tt, c,
                             off + sub * 128:off + (sub + 1) * 128],
                        wv[:, c, :],
                        start=(c == 0), stop=(c == NCHUNK - 1))
                ge = n1_pool.tile([128, NHEADS], F32, tag="ge")
                nc.scalar.activation(out=ge[:], in_=vps[:, 256:260],
                                     func=AF.Exp, scale=-1.0)
                nc.vector.tensor_scalar_add(ge[:], ge[:], 1.0)
                gf = n1_pool.tile([128, NHEADS], F32, tag="gf")
                nc.vector.reciprocal_approx_fast(out=gf[:], in_=ge[:])
                gb16 = n1_pool.tile([128, NHEADS], BF16, tag="gb16")
                nc.vector.tensor_copy(gb16[:], gf[:])
                gap = gb16[:]
                gb = bass.AP(tensor=gap.tensor, offset=gap.offset,
                             ap=[list(gap.ap[0]), [1, NHEADS], [0, HD]])
                tmp = n1_pool.tile([128, NHEADS, HD], BF16, tag="vtmp")
                nc.gpsimd.tensor_mul(
                    tmp[:],
                    vesb[:, stg, :].rearrange("p (h d) -> p h d", h=NHEADS),
                    gb)
                nc.vector.tensor_add(
                    Vh[:, stg, :, 0:64],
                    vps[:, 0:256].rearrange("p (h d) -> p h d", h=NHEADS),
                    tmp[:])

            # rms-norm statistics
            for blk in range(NHEADS):
                nc.tensor.matmul(ss8z[0:8, 0:tlen], onesEt[:, blk, :],
                                 sq[:, blk, 0:tlen],
                                 start=False, stop=(blk == NHEADS - 1))
            lnss = n1_pool.tile([8, TTILE], F32, tag="lnss")
            nc.scalar.activation(out=lnss[:, 0:tlen], in_=ss8z[0:8, 0:tlen],
                                 func=AF.Ln, scale=1.0 / HD, bias=epsb[:])
            rstd8 = n1_pool.tile([8, TTILE], BF16, tag="rstd8")
            nc.scalar.activation(out=rstd8[:, 0:tlen], in_=lnss[:, 0:tlen],
                                 func=AF.Exp, scale=-0.5)
            age = n1_pool.tile([NHEADS, TTILE], F32, tag="age")
            nc.scalar.activation(out=age[:, 0:tlen], in_=ss8z[32:36, 0:tlen],
                                 func=AF.Exp, scale=-1.0)
            nc.vector.tensor_scalar_add(agrow1[:, t0:t0 + tlen],
                                        age[:, 0:tlen], 1.0)

            # broadcast rstd to per-dim rows, all heads into one sbuf tile
            rstdb = sb_pool.tile([128, NHEADS, TTILE], BF16, tag="rstdb")
            for blk in range(NHEADS):
                rbps = rbps_pool.tile([128, TTILE], F32, tag="rb")
                nc.tensor.matmul(rbps[:, 0:tlen], e8sel[:, blk, :],
                                 rstd8[:, 0:tlen], start=True, stop=True)
                nc.scalar.activation(out=rstdb[:, blk, 0:tlen],
                                     in_=rbps[:, 0:tlen], func=AF.Copy,
                                     scale=1.0)

            # apply rstd into persistent Q^/K^ (KEY_OFFSET on rows 32:64)
            nc.vector.tensor_mul(Qh[0:64, :, t0:t0 + tlen],
                                 rotr[0:64, :, 0:tlen],
                                 rstdb[0:64, :, 0:tlen])
            nc.vector.tensor_mul(Kh[0:32, :, t0:t0 + tlen],
                                 rotr[64:96, :, 0:tlen],
                                 rstdb[64:96, :, 0:tlen])
            w = tlen if t0 + tlen < T else tlen - 1
            nc.vector.tensor_mul(Kh[32:64, :, t0 + 1:t0 + 1 + w],
                                 rotr[96:128, :, 0:w],
                                 rstdb[96:128, :, 0:w])
            if t0 == 0:
                nc.vector.tensor_mul(Kh[32:64, :, 0:1],
                                     rotr[96:128, :, 0:1],
                                     rstdb[96:128, :, 0:1])

    # =========== Phase 2 (attention + interleaved o-proj) ===========
    ypool = ctx.enter_context(tc.tile_pool(name="ylate", bufs=1))
    y01 = ypool.tile([128, T], BF16)
    y23 = ypool.tile([128, T], BF16)

    oq = [nc.sync, nc.gpsimd]

    def oproj(ti, ops_pool, osb_pool):
        tt0 = ti * 128
        for eh in range(2):
            ops = ops_pool.tile([128, 512], F32, tag="o")
            nc.tensor.matmul(ops[:], y01[:, tt0:tt0 + 128],
                             wo[:, 0, eh * 512:(eh + 1) * 512],
                             start=True, stop=False)
            nc.tensor.matmul(ops[:], y23[:, tt0:tt0 + 128],
                             wo[:, 1, eh * 512:(eh + 1) * 512],
                             start=False, stop=True)
            osb = osb_pool.tile([128, 512], BF16, tag="osb")
            if eh == 0:
                nc.vector.tensor_scalar_mul(osb[:], ops[:], 1.0)
            else:
                nc.scalar.activation(out=osb[:], in_=ops[:], func=AF.Copy,
                                     scale=1.0)
            oq[eh].dma_start(
                dr["out"][tt0:tt0 + 128, eh * 512:(eh + 1) * 512], osb[:])

    with tc.tile_pool(name="p2s", bufs=2, space="PSUM") as sps_pool, \
         tc.tile_pool(name="p2y", bufs=1, space="PSUM") as yps_pool, \
         tc.tile_pool(name="p3ps", bufs=2, space="PSUM") as ops_pool, \
         tc.tile_pool(name="p3sb", bufs=3) as osb_pool, \
         tc.tile_pool(name="p2p", bufs=6) as pt_pool, \
         tc.tile_pool(name="p2sc", bufs=2) as sc_pool:
        bps_pool = ops_pool
        pending = []
        onext = 0
        for (a, b_, kts) in spans:
            N = b_ - a
            ycps = []
            l4 = sc_pool.tile([NHEADS, 512], BF16, tag="l4")
            for h in range(NHEADS):
                yps = yps_pool.tile([65, 512], F32, tag=f"y{h}")
                for ki, (ka, kn, causal, qhi) in enumerate(kts):
                    w0 = max(0, ka - a)
                    sps = sps_pool.tile([128, 512], F32, tag="s")
                    nc.tensor.matmul(sps[0:kn, w0:qhi],
                                     Kh[:, h, ka:ka + kn],
                                     Qh[:, h, a + w0:a + qhi],
                                     start=True, stop=True)
                    pt = pt_pool.tile([128, 512], BF16, tag="p")
                    nc.scalar.activation(out=pt[0:kn, w0:qhi],
                                         in_=sps[0:kn, w0:qhi],
                                         func=AF.Exp, scale=alpha)
                    if causal:
                        bw = min(qhi, ka + kn - a) - w0
                        if bw > 0:
                            nc.gpsimd.affine_select(
                                out=pt[0:kn, w0:w0 + bw],
                                in_=pt[0:kn, w0:w0 + bw],
                                compare_op=mybir.AluOpType.is_ge,
                                fill=0.0, base=a + w0 - ka,
                                pattern=[[1, bw]], channel_multiplier=-1)
                    nc.tensor.matmul(yps[:, w0:qhi],
                                     Vh[0:kn, ka // 128, h, :],
                                     pt[0:kn, w0:qhi],
                                     start=(ki == 0), stop=(ki == len(kts) - 1))
                # one 2x psum->bf16 copy per head serves both the softmax
                # denominator row and the bf16 scaling muls below; doing it
                # here (not after all heads) spreads DVE load across the span
                ycp = sc_pool.tile([65, 512], BF16, tag=f"ycp{h}")
                nc.vector.tensor_scalar_mul(ycp[:, 0:N], yps[:, 0:N], 1.0)
                ycps.append(ycp)
                nc.sync.dma_start(l4[h:h + 1, 0:N], ycp[64:65, 0:N])
                # paced o-proj filler placed here: the epilogue window is
                # where the PE would otherwise idle (DVE/DMA-bound)
                if pending:
                    oproj(pending.pop(0), ops_pool, osb_pool)
            mm = sc_pool.tile([NHEADS, 512], F32, tag="mm")
            nc.vector.tensor_mul(mm[:, 0:N], l4[:, 0:N], agrow1[:, a:b_])
            scf = sc_pool.tile([NHEADS, 512], F32, tag="scf")
            nc.vector.reciprocal_approx_fast(out=scf[:, 0:N], in_=mm[:, 0:N])
            sc4 = sc_pool.tile([NHEADS, 512], BF16, tag="sc")
            nc.vector.tensor_copy(sc4[:, 0:N], scf[:, 0:N])
            for pr, ytile in ((0, y01), (1, y23)):
                sbcps = bps_pool.tile([128, 512], F32, tag="o")
                nc.tensor.matmul(sbcps[:, 0:N], e4a[:] if pr == 0 else e4b[:],
                                 sc4[:, 0:N], start=True, stop=True)
                nc.vector.tensor_mul(ytile[0:64, a:b_],
                                     ycps[2 * pr][0:64, 0:N],
                                     sbcps[0:64, 0:N])
                nc.vector.tensor_mul(ytile[64:128, a:b_],
                                     ycps[2 * pr + 1][0:64, 0:N],
                                     sbcps[64:128, 0:N])
            pending.extend(range(onext, b_ // 128))
            onext = b_ // 128
        for ti in pending:
            oproj(ti, ops_pool, osb_pool)


_CACHE = {}
TRACE = False       # set by test harness to capture an NTFF profile
LAST_RESULT = None  # BassKernelResults of the most recent run


def _get_program(key, spans, alpha):
    if key in _CACHE:
        return _CACHE[key]
    nc = bacc.Bacc("TRN2", target_bir_lowering=False, debug=False)
    dr = {}

    def di(name, shape, dt=F32):
        dr[name] = nc.dram_tensor(name, shape, dt, kind="ExternalInput").ap()

    di("xT", [128, NTT * NCHUNK * TTILE], BF16)
    di("veb", [128, (T // 128) * 256], BF16)
    di("wqk", [128, NCHUNK * 512], BF16)
    di("wv", [128, NCHUNK * 260], BF16)
    di("wga36", [128, NCHUNK * 36], BF16)
    di("wo", [128, 2 * 1024], BF16)
    di("cdup", [128, T], BF16)
    di("s2dup", [128, T], BF16)
    di("qaug", [AUG, T], BF16)
    di("kaug", [AUG, T], BF16)
    di("onesEt", [128, NHEADS * 8], BF16)
    di("e8sel", [8, NHEADS * 128], BF16)
    di("e4", [2, 4, 128], BF16)
    dr["out"] = nc.dram_tensor("out", [T, D], BF16,
                               kind="ExternalOutput").ap()
    with tile.TileContext(nc) as tc:
        build_kernel(tc, dr, spans, alpha)
    nc.compile()
    _CACHE[key] = nc
    return nc


def kernel(x, ve, sa_lambdas, cos, sin, qkvo_w, attn_gate_w, ve_gate_w,
           attn_scale, docs):
    x = np.asarray(x, dtype=np.float32)
    ve = np.asarray(ve, dtype=np.float32)
    sa_lambdas = np.asarray(sa_lambdas, dtype=np.float32)
    cos = np.asarray(cos, dtype=np.float32)
    sin = np.asarray(sin, dtype=np.float32)
    qkvo_w = np.asarray(qkvo_w, dtype=np.float32)
    attn_gate_w = np.asarray(attn_gate_w, dtype=np.float32)
    ve_gate_w = np.asarray(ve_gate_w, dtype=np.float32)
    docs = np.asarray(docs, dtype=np.int32)
    alpha = float(np.asarray(attn_scale))

    segs = []
    s = 0
    for t in range(1, T + 1):
        if t == T or docs[t] != docs[t - 1]:
            segs.append((s, t))
            s = t
    spans = build_spans(segs)
    nc = _get_program((tuple(segs), alpha), spans, alpha)

    lam0, lam1 = float(sa_lambdas[0]), float(sa_lambdas[1])

    cosT = np.ascontiguousarray(cos.T)
    sinT = np.ascontiguousarray(sin.T)
    cblk = np.concatenate([cosT[0:16], cosT[0:16], cosT[16:32], cosT[16:32]],
                          axis=0)
    sblk = np.concatenate([-sinT[0:16], sinT[0:16], -sinT[16:32],
                           sinT[16:32]], axis=0)
    cdup = np.tile(cblk, (2, 1)).astype(BF)
    s2dup = np.tile(sblk, (2, 1)).astype(BF)
    onehot = (docs[None, :] == np.arange(NDOC)[:, None]).astype(np.float32)
    kaug = np.concatenate([onehot, np.ones((1, T), np.float32)],
                          axis=0).astype(BF)
    qaug = np.concatenate(
        [(BIG / alpha) * onehot, -(BIG / alpha) * np.ones((1, T), np.float32)],
        axis=0).astype(BF)
    onesEt = np.zeros((128, NHEADS, 8), np.float32)
    e8sel = np.zeros((8, NHEADS, 128), np.float32)
    for b in range(NHEADS):
        onesEt[0:64, b, 2 * b] = 1.0
        onesEt[64:128, b, 2 * b + 1] = 1.0
        e8sel[2 * b, b, 0:64] = 1.0
        e8sel[2 * b + 1, b, 64:128] = 1.0
    onesEt = onesEt.reshape(128, -1).astype(BF)
    e8sel = e8sel.reshape(8, -1).astype(BF)
    e4 = np.zeros((2, 4, 128), np.float32)
    e4[0, 0, 0:64] = 1.0
    e4[0, 1, 64:128] = 1.0
    e4[1, 2, 0:64] = 1.0
    e4[1, 3, 64:128] = 1.0
    e4 = e4.astype(BF)

    Wq, Wk, Wv, Wo = (qkvo_w[0:D], qkvo_w[D:2 * D], qkvo_w[2 * D:3 * D],
                      qkvo_w[3 * D:4 * D])

    in_maps = []
    for core in range(8):
        b = core // HGROUPS
        hg = core % HGROUPS
        heads = list(range(hg * NHEADS, (hg + 1) * NHEADS))
        perm = np.r_[0:16, 32:48, 16:32, 48:64]
        blocks = []
        for h in heads:
            blocks.append(lam0 * Wq[h * HD:(h + 1) * HD][perm].T)
            blocks.append(lam0 * Wk[h * HD:(h + 1) * HD][perm].T)
        wqk = np.concatenate(blocks, axis=1).astype(np.float32)
        wqk = np.ascontiguousarray(
            wqk.reshape(NCHUNK, 128, 512).transpose(1, 0, 2)
            .reshape(128, -1)).astype(BF)
        wv_cols = [lam0 * Wv[h * HD:(h + 1) * HD].T for h in heads]
        wv_cols.append(ve_gate_w[heads].T)
        wv = np.concatenate(wv_cols, axis=1).astype(np.float32)
        wv = np.ascontiguousarray(
            wv.reshape(NCHUNK, 128, 260).transpose(1, 0, 2)
            .reshape(128, -1)).astype(BF)
        wga36 = np.zeros((D, 36), np.float32)
        wga36[:, 32:36] = attn_gate_w[heads].T
        wga36 = np.ascontiguousarray(
            wga36.reshape(NCHUNK, 128, 36).transpose(1, 0, 2)
            .reshape(128, -1)).astype(BF)
        wo = (lam1 * Wo[:, hg * 256:(hg + 1) * 256].T).astype(np.float32)
        wo = np.ascontiguousarray(
            wo.reshape(2, 128, 1024).transpose(1, 0, 2)
            .reshape(128, -1)).astype(BF)
        xTn = x[b].T.astype(np.float32)  # [D, T]
        xT = np.ascontiguousarray(
            xTn.reshape(NCHUNK, 128, NTT, TTILE).transpose(1, 2, 0, 3)
            .reshape(128, -1)).astype(BF)
        veb = np.ascontiguousarray(
            (VE_GATE_SCALE * ve[b, :, hg * 256:(hg + 1) * 256])
            .reshape(T // 128, 128, 256).transpose(1, 0, 2)
            .reshape(128, -1)).astype(BF)
        in_maps.append({
            "xT": xT, "veb": veb, "wqk": wqk, "wv": wv, "wga36": wga36,
            "wo": wo, "cdup": cdup, "s2dup": s2dup, "qaug": qaug,
            "kaug": kaug, "onesEt": onesEt, "e8sel": e8sel, "e4": e4,
        })

    global LAST_RESULT
    res = run_bass_kernel_spmd(nc, in_maps, list(range(8)), trace=TRACE)
    LAST_RESULT = res
    out = np.zeros((B, T, D), dtype=np.float32)
    for core in range(8):
        out[core // HGROUPS] += res.results[core]["out"].astype(np.float32)
    return out
